# revision 11
# baseline (speedup 1.0000x reference)
"""Bahdanau-style ragged-sequence attention on 8 TRN2 NeuronCores.

Data-parallel over batch: each core owns 4 batches of
encoder_out [4, 2048, 1024], computes masked-softmax attention context
[4, 1024]; host concatenates the 8 shards.

Per-core pipeline (per batch b, per s-half h of 1024 positions):
  1. SWDGE cast-DMA: enc[b, h] f32 HBM -> bf16 SBUF [128, 8, 1024] (s=t*128+p)
  2. xbar DMA-transpose: -> encT bf16 [128(f%128), 8(f//128), 8(t), 128(q)]
  3. PE: M^T[d, s] = sum_f W_e[d, f] enc[s, f]  (bf16, fp32 PSUM accum)
     ACT: e^T = tanh(M^T + c_b)  (c_b = W_h h_b + attn_b, precomputed on PE)
  4. PE: scoresT[s-slice, 1] = e^T_slice.T @ v  -> scores laid [128, 8]
  5. ACT exp (no max-sub needed: |score| <= |v|_1 ~ 12), DVE mask -> u bf16
  6. PE: ctx_unnorm[1, 1024] += u_t.T @ enc_t ; U[1,1] += u_t.T @ ones
  7. DVE: ctx = ctx_unnorm / U -> DMA out.

v_b is skipped: softmax is invariant to a uniform logit shift, and masked
positions are exact zeros via the mask multiply.
"""
import sys

sys.path.insert(0, '/opt/trn_rl_repo')

from contextlib import ExitStack

import numpy as np
import ml_dtypes

import concourse.bass as bass
import concourse.tile as tile
from concourse import bacc, mybir

B, S, F, D = 32, 2048, 1024, 512
NCORES = 8
BL = B // NCORES          # batches per core
NH = 4                    # s-chunks per batch
SH = S // NH              # 1024 positions per half
NT = SH // 128            # 8 s-tiles per half
NJ = F // 128             # 8 f-chunks
ND = D // 128             # 4 d-chunks
NSPAN = SH // 512         # 2 matmul spans of 512 per half

f32 = mybir.dt.float32
bf16 = mybir.dt.bfloat16


def build_nc():
    nc = bacc.Bacc('TRN2', target_bir_lowering=False, debug=False)

    enc_d = nc.dram_tensor("enc", [BL, S, F], f32, kind="ExternalInput")
    weT_d = nc.dram_tensor("weT", [F, D], bf16, kind="ExternalInput")
    whT_d = nc.dram_tensor("whT", [D, D], f32, kind="ExternalInput")
    hidT_d = nc.dram_tensor("hidT", [D, BL], f32, kind="ExternalInput")
    battn_d = nc.dram_tensor("battn", [D], f32, kind="ExternalInput")
    vw_d = nc.dram_tensor("vw", [D], bf16, kind="ExternalInput")
    lens_d = nc.dram_tensor("lens", [128, BL], f32, kind="ExternalInput")
    iota_d = nc.dram_tensor("iota", [128, NH * NT], f32, kind="ExternalInput")
    ones_d = nc.dram_tensor("ones", [128, 1], f32, kind="ExternalInput")
    out_d = nc.dram_tensor("out", [BL, F], f32, kind="ExternalOutput")

    with tile.TileContext(nc) as tc, ExitStack() as ctx:
        wpool = ctx.enter_context(tc.tile_pool(name="weights", bufs=1))
        encp = ctx.enter_context(tc.tile_pool(name="encp", bufs=6))
        enctp = ctx.enter_context(tc.tile_pool(name="enctp", bufs=6))
        etp = ctx.enter_context(tc.tile_pool(name="etp", bufs=3))
        smallp = ctx.enter_context(tc.tile_pool(name="small", bufs=4))
        psM = ctx.enter_context(tc.tile_pool(name="psM", bufs=3, space="PSUM"))
        psS = ctx.enter_context(tc.tile_pool(name="psS", bufs=2, space="PSUM"))
        psC = ctx.enter_context(tc.tile_pool(name="psC", bufs=1, space="PSUM"))
        psU = ctx.enter_context(tc.tile_pool(name="psU", bufs=1, space="PSUM"))

        # ---- one-time setup ----
        weT = wpool.tile([128, NJ, D], bf16)
        nc.sync.dma_start(weT[:], weT_d.ap().rearrange("(j p) d -> p j d", p=128))
        whT = wpool.tile([128, ND, D], f32)
        nc.sync.dma_start(whT[:], whT_d.ap().rearrange("(j p) d -> p j d", p=128))
        hidT = wpool.tile([128, ND, BL], f32)
        nc.sync.dma_start(hidT[:], hidT_d.ap().rearrange("(j p) b -> p j b", p=128))
        b_sb = wpool.tile([128, ND], f32)
        nc.sync.dma_start(b_sb[:], battn_d.ap().rearrange("(c p) -> p c", p=128))
        v_sb = wpool.tile([128, ND], bf16)
        nc.sync.dma_start(v_sb[:], vw_d.ap().rearrange("(c p) -> p c", p=128))
        len_sb = wpool.tile([128, BL], f32)
        nc.sync.dma_start(len_sb[:], lens_d.ap())
        iota_sb = wpool.tile([128, NH * NT], f32)
        nc.sync.dma_start(iota_sb[:], iota_d.ap())
        ones_sb = wpool.tile([128, 1], f32)
        nc.sync.dma_start(ones_sb[:], ones_d.ap())

        # c[d, b] = (W_h @ h_b + attn_b) laid [128, dc, b]
        ps_c = psS.tile([128, ND * BL], f32, tag="ps_s")
        for dc in range(ND):
            for j in range(ND):
                nc.tensor.matmul(
                    ps_c[:, dc * BL:(dc + 1) * BL],
                    whT[:, j, dc * 128:(dc + 1) * 128],
                    hidT[:, j, :],
                    start=(j == 0), stop=(j == ND - 1))
        c_sb = wpool.tile([128, ND, BL], f32)
        for dc in range(ND):
            nc.scalar.activation(
                c_sb[:, dc, :], ps_c[:, dc * BL:(dc + 1) * BL],
                mybir.ActivationFunctionType.Identity, bias=b_sb[:, dc:dc + 1])

        # ---- main loop ----
        for b in range(BL):
            ctx_ps = None
            u_ps = None
            for h in range(NH):
                # 1. cast-load this half: [SH, F] f32 -> [128, NT, F] bf16
                enc_bf = encp.tile([128, NT, F], bf16)
                src = enc_d.ap()[b, h * SH:(h + 1) * SH, :]
                nc.gpsimd.dma_start(
                    enc_bf[:], src.rearrange("(t p) f -> p t f", p=128))

                # 2. xbar transpose (one call for the whole half):
                # encT[p, t, j, q] = enc[t*128+q, j*128+p]
                encT = enctp.tile([128, NT, NJ, 128], bf16)
                nc.sync.dma_start_transpose(encT[:, :, :, :], enc_bf[:, :, :])

                # 3. scores matmul + tanh -> eT [128, ND, SH] bf16
                eT = etp.tile([128, ND, SH], bf16)
                for n in range(NSPAN):
                    tsl = slice(n * 4, (n + 1) * 4)
                    for dc in range(ND):
                        pm = psM.tile([128, 512], f32)
                        for j in range(NJ):
                            nc.tensor.matmul(
                                pm[:],
                                weT[:, j, dc * 128:(dc + 1) * 128],
                                encT[:, tsl, j, :],
                                start=(j == 0), stop=(j == NJ - 1))
                        nc.scalar.activation(
                            eT[:, dc, n * 512:(n + 1) * 512], pm[:],
                            mybir.ActivationFunctionType.Tanh,
                            bias=c_sb[:, dc, b:b + 1])

                # 4. scoresT [128, NT]: col t = e^T slice.T @ v
                ps_s = psS.tile([128, NT], f32, tag="ps_s")
                for t in range(NT):
                    for dc in range(ND):
                        nc.tensor.matmul(
                            ps_s[:, t:t + 1],
                            eT[:, dc, t * 128:(t + 1) * 128],
                            v_sb[:, dc:dc + 1],
                            start=(dc == 0), stop=(dc == ND - 1))

                # 5. u = exp(scores) * mask  (bf16)
                mask = smallp.tile([128, NT], f32, tag="mask")
                nc.vector.tensor_scalar(
                    mask[:], iota_sb[:, h * NT:(h + 1) * NT],
                    len_sb[:, b:b + 1], None, mybir.AluOpType.is_lt)
                u_f = smallp.tile([128, NT], f32, tag="uf")
                nc.scalar.activation(u_f[:], ps_s[:],
                                     mybir.ActivationFunctionType.Exp)
                u_bf = smallp.tile([128, NT], bf16, tag="ubf")
                nc.vector.tensor_tensor(u_bf[:], u_f[:], mask[:],
                                        mybir.AluOpType.mult)

                # 6. context accumulation across both halves; U via DVE
                if h == 0:
                    ctx_ps = psC.tile([1, F], f32)
                    u_ps = psU.tile([1, 1], f32)
                    us2 = smallp.tile([128, NH], f32, tag="us2")
                nc.vector.tensor_reduce(us2[:, h:h + 1], u_bf[:],
                                        mybir.AxisListType.X,
                                        mybir.AluOpType.add)
                for t in range(NT):
                    first = (h == 0 and t == 0)
                    last = (h == NH - 1 and t == NT - 1)
                    nc.tensor.matmul(ctx_ps[0:1, 0:512], u_bf[:, t:t + 1],
                                     enc_bf[:, t, 0:512],
                                     start=first, stop=last,
                                     skip_group_check=True)
                    nc.tensor.matmul(ctx_ps[0:1, 512:1024], u_bf[:, t:t + 1],
                                     enc_bf[:, t, 512:1024],
                                     start=first, stop=last,
                                     skip_group_check=True)

            # 7. finalize batch: U = sum over partitions of (us2[:,0]+us2[:,1])
            uscol = smallp.tile([128, 1], f32, tag="uscol")
            nc.vector.tensor_reduce(uscol[:], us2[:],
                                    mybir.AxisListType.X,
                                    mybir.AluOpType.add)
            nc.tensor.matmul(u_ps[:], uscol[:], ones_sb[:], start=True, stop=True)
            rU = smallp.tile([1, 1], f32, tag="rU")
            nc.vector.reciprocal(rU[:], u_ps[0:1, 0:1])
            ctx_sb = smallp.tile([1, F], f32, tag="ctx")
            nc.vector.tensor_scalar(ctx_sb[:], ctx_ps[:], rU[0:1, 0:1], None,
                                    mybir.AluOpType.mult)
            nc.sync.dma_start(out_d.ap()[b:b + 1, :], ctx_sb[:])

    nc.compile()
    return nc


def make_in_maps(encoder_out, hidden, lengths, attn_w, attn_b, v_w):
    encoder_out = np.ascontiguousarray(np.asarray(encoder_out, dtype=np.float32))
    hidden = np.asarray(hidden, dtype=np.float32)
    lengths = np.asarray(lengths).astype(np.float32)
    attn_w = np.asarray(attn_w, dtype=np.float32)
    attn_b = np.asarray(attn_b, dtype=np.float32)
    v_w = np.asarray(v_w, dtype=np.float32)

    weT = np.ascontiguousarray(attn_w[:, D:].T).astype(ml_dtypes.bfloat16)
    whT = np.ascontiguousarray(attn_w[:, :D].T)
    vw = v_w[0].astype(ml_dtypes.bfloat16)
    iota = np.ascontiguousarray(
        np.arange(S, dtype=np.float32).reshape(NH * NT, 128).T)
    ones = np.ones((128, 1), dtype=np.float32)

    in_maps = []
    for i in range(NCORES):
        sl = slice(i * BL, (i + 1) * BL)
        in_maps.append({
            "enc": np.ascontiguousarray(encoder_out[sl]),
            "weT": weT,
            "whT": whT,
            "hidT": np.ascontiguousarray(hidden[sl].T),
            "battn": attn_b,
            "vw": vw,
            "lens": np.ascontiguousarray(
                np.broadcast_to(lengths[sl], (128, BL))),
            "iota": iota,
            "ones": ones,
        })
    return in_maps


_cached_nc = None


def kernel(encoder_out, hidden, lengths, attn_w, attn_b, v_w, v_b):
    global _cached_nc
    from concourse.bass_utils import run_bass_kernel_spmd

    if _cached_nc is None:
        _cached_nc = build_nc()
    in_maps = make_in_maps(encoder_out, hidden, lengths, attn_w, attn_b, v_w)
    res = run_bass_kernel_spmd(_cached_nc, in_maps, core_ids=list(range(NCORES)))
    out = np.concatenate([res.results[i]["out"] for i in range(NCORES)], axis=0)
    return out.astype(np.float32)


# revision 20
# speedup vs baseline: 1.0193x; 1.0193x over previous
"""Bahdanau-style ragged-sequence attention on 8 TRN2 NeuronCores.

Data-parallel over batch: each core owns 4 batches of
encoder_out [4, 2048, 1024], computes masked-softmax attention context
[4, 1024]; host concatenates the 8 shards.

Per-core pipeline (per batch b, per s-chunk h of SH positions):
  1. SWDGE cast-DMA: enc[b, h] f32 HBM -> bf16 SBUF [128, NT, F] (s=t*128+p)
  2. encT build, split to balance DMA vs PE:
     - xbar DMA-transpose for s-tiles 0..XB_T-1 (one call)
     - PE transpose-mode + ACT evacuation for the rest
     encT[p, t, j, q] = enc[t*128+q, j*128+p]
  3. PE: M^T[d, s] = sum_f W_e[d, f] enc[s, f]  (bf16, fp32 PSUM accum)
     ACT: e^T = tanh(M^T + c_b)  (c_b = W_h h_b + attn_b, precomputed on PE)
  4. PE: scores row [1, s] = sum_dc v_dc^T e^T_dc   (form A)
  5. ACT exp (no max-sub needed: |score| <= |v|_1 ~ 12); DVE mask-mul -> u
     row bf16; DVE reduce -> U contribution
  6. PE rank-1 broadcast: ubc = ones ⊗ u_row (K=1 matmul), ACT evac to bf16
     DVE tensor_tensor_reduce: ctxT[:, j] += sum_s encT[:, :, j, :] * ubc
  7. DVE: ctxT *= 1/U (U bcast via tiny K=1 matmul) -> DMA out.

v_b is skipped: softmax is invariant to a uniform logit shift, and masked
positions are exact zeros via the mask multiply.
"""
import sys

sys.path.insert(0, '/opt/trn_rl_repo')

from contextlib import ExitStack

import numpy as np
import ml_dtypes

import concourse.bass as bass
import concourse.tile as tile
from concourse import bacc, mybir

B, S, F, D = 32, 2048, 1024, 512
NCORES = 8
BL = B // NCORES          # batches per core
NH = 2                    # s-chunks per batch
SH = S // NH              # positions per chunk
NT = SH // 128            # s-tiles per chunk
NJ = F // 128             # f-chunks
ND = D // 128             # d-chunks
NSPAN = SH // 512         # matmul spans of 512 per chunk
XB_T = 6                  # s-tiles per chunk transposed via xbar DMA
                          # (the rest go through PE transpose-mode)

f32 = mybir.dt.float32
bf16 = mybir.dt.bfloat16


def build_nc():
    nc = bacc.Bacc('TRN2', target_bir_lowering=False, debug=False)

    enc_d = nc.dram_tensor("enc", [BL, S, F], f32, kind="ExternalInput")
    weT_d = nc.dram_tensor("weT", [F, D], bf16, kind="ExternalInput")
    whT_d = nc.dram_tensor("whT", [D, D], f32, kind="ExternalInput")
    hidT_d = nc.dram_tensor("hidT", [D, BL], f32, kind="ExternalInput")
    battn_d = nc.dram_tensor("battn", [D], f32, kind="ExternalInput")
    vw_d = nc.dram_tensor("vw", [D], bf16, kind="ExternalInput")
    lens_d = nc.dram_tensor("lens", [1, BL], f32, kind="ExternalInput")
    iota_d = nc.dram_tensor("iota_row", [1, S], f32, kind="ExternalInput")
    onesr_d = nc.dram_tensor("ones_row", [1, 128], bf16, kind="ExternalInput")
    onesrf_d = nc.dram_tensor("ones_row_f", [1, 128], f32, kind="ExternalInput")
    ident_d = nc.dram_tensor("ident", [128, 128], bf16, kind="ExternalInput")
    identf_d = nc.dram_tensor("identf", [128, 128], f32, kind="ExternalInput")
    out_d = nc.dram_tensor("out", [BL, F], f32, kind="ExternalOutput")

    with tile.TileContext(nc) as tc, ExitStack() as ctx:
        wpool = ctx.enter_context(tc.tile_pool(name="weights", bufs=1))
        encp = ctx.enter_context(tc.tile_pool(name="encp", bufs=3))
        enctp = ctx.enter_context(tc.tile_pool(name="enctp", bufs=3))
        etp = ctx.enter_context(tc.tile_pool(name="etp", bufs=3))
        ubcp = ctx.enter_context(tc.tile_pool(name="ubcp", bufs=2))
        dumpp = ctx.enter_context(tc.tile_pool(name="dump", bufs=1))
        smallp = ctx.enter_context(tc.tile_pool(name="small", bufs=4))
        psM = ctx.enter_context(tc.tile_pool(name="psM", bufs=2, space="PSUM"))
        psS = ctx.enter_context(tc.tile_pool(name="psS", bufs=2, space="PSUM"))
        psB = ctx.enter_context(tc.tile_pool(name="psB", bufs=2, space="PSUM"))
        psT = ctx.enter_context(tc.tile_pool(name="psT", bufs=2, space="PSUM"))

        # ---- one-time setup ----
        weT = wpool.tile([128, NJ, D], bf16)
        nc.sync.dma_start(weT[:], weT_d.ap().rearrange("(j p) d -> p j d", p=128))
        whT = wpool.tile([128, ND, D], f32)
        nc.sync.dma_start(whT[:], whT_d.ap().rearrange("(j p) d -> p j d", p=128))
        hidT = wpool.tile([128, ND, BL], f32)
        nc.sync.dma_start(hidT[:], hidT_d.ap().rearrange("(j p) b -> p j b", p=128))
        b_sb = wpool.tile([128, ND], f32)
        nc.sync.dma_start(b_sb[:], battn_d.ap().rearrange("(c p) -> p c", p=128))
        v_sb = wpool.tile([128, ND], bf16)
        nc.sync.dma_start(v_sb[:], vw_d.ap().rearrange("(c p) -> p c", p=128))
        len_sb = wpool.tile([1, BL], f32)
        nc.sync.dma_start(len_sb[:], lens_d.ap())
        iota_sb = wpool.tile([1, S], f32)
        nc.sync.dma_start(iota_sb[:], iota_d.ap())
        onesr_sb = wpool.tile([1, 128], bf16)
        nc.sync.dma_start(onesr_sb[:], onesr_d.ap())
        onesrf_sb = wpool.tile([1, 128], f32)
        nc.sync.dma_start(onesrf_sb[:], onesrf_d.ap())
        ident_sb = wpool.tile([128, 128], bf16)
        nc.sync.dma_start(ident_sb[:], ident_d.ap())
        identf_sb = wpool.tile([128, 128], f32)
        nc.sync.dma_start(identf_sb[:], identf_d.ap())

        # c[d, b] = (W_h @ h_b + attn_b) laid [128, dc, b]
        ps_c = psS.tile([128, ND * BL], f32, tag="ps_s")
        for dc in range(ND):
            for j in range(ND):
                nc.tensor.matmul(
                    ps_c[:, dc * BL:(dc + 1) * BL],
                    whT[:, j, dc * 128:(dc + 1) * 128],
                    hidT[:, j, :],
                    start=(j == 0), stop=(j == ND - 1))
        c_sb = wpool.tile([128, ND, BL], f32)
        for dc in range(ND):
            nc.scalar.activation(
                c_sb[:, dc, :], ps_c[:, dc * BL:(dc + 1) * BL],
                mybir.ActivationFunctionType.Identity, bias=b_sb[:, dc:dc + 1])

        # ---- main loop ----
        for b in range(BL):
            ctxT_acc = None
            Usc = smallp.tile([1, NH], f32, tag="Usc")
            for h in range(NH):
                # 1. cast-load this chunk: [SH, F] f32 -> [128, NT, F] bf16
                enc_bf = encp.tile([128, NT, F], bf16)
                src = enc_d.ap()[b, h * SH:(h + 1) * SH, :]
                nc.gpsimd.dma_start(
                    enc_bf[:], src.rearrange("(t p) f -> p t f", p=128))

                # 2. encT[p, t, j, q] = enc[t*128+q, j*128+p]
                encT = enctp.tile([128, NT, NJ, 128], bf16)
                nc.sync.dma_start_transpose(encT[:, 0:XB_T, :, :],
                                            enc_bf[:, 0:XB_T, :])
                for t in range(XB_T, NT):
                    for j in range(NJ):
                        ptr = psT.tile([128, 128], bf16, tag="ptr")
                        nc.tensor.transpose(
                            ptr[:], enc_bf[:, t, j * 128:(j + 1) * 128],
                            ident_sb[:])
                        nc.scalar.copy(encT[:, t, j, :], ptr[:])

                # 3. scores matmul + tanh -> eT [128, ND, SH] bf16
                eT = etp.tile([128, ND, SH], bf16)
                for n in range(NSPAN):
                    tsl = slice(n * 4, (n + 1) * 4)
                    for dc in range(ND):
                        pm = psM.tile([128, 512], f32)
                        for j in range(NJ):
                            nc.tensor.matmul(
                                pm[:],
                                weT[:, j, dc * 128:(dc + 1) * 128],
                                encT[:, tsl, j, :],
                                start=(j == 0), stop=(j == NJ - 1))
                        nc.scalar.activation(
                            eT[:, dc, n * 512:(n + 1) * 512], pm[:],
                            mybir.ActivationFunctionType.Tanh,
                            bias=c_sb[:, dc, b:b + 1])

                # 4.-6. per 512-span: scores row, exp, mask, broadcast
                ubc = ubcp.tile([128, NT, 128], bf16)
                u_row = smallp.tile([1, SH], bf16, tag="u_row")
                for n in range(NSPAN):
                    ps_row = psS.tile([1, 512], f32, tag="ps_s")
                    for dc in range(ND):
                        nc.tensor.matmul(
                            ps_row[:], v_sb[:, dc:dc + 1],
                            eT[:, dc, n * 512:(n + 1) * 512],
                            start=(dc == 0), stop=(dc == ND - 1))
                    uf = smallp.tile([1, 512], f32, tag="uf")
                    nc.scalar.activation(uf[:], ps_row[:],
                                         mybir.ActivationFunctionType.Exp)
                    mrow = smallp.tile([1, 512], f32, tag="mrow")
                    off = h * SH + n * 512
                    nc.vector.tensor_scalar(
                        mrow[:], iota_sb[0:1, off:off + 512],
                        len_sb[0:1, b:b + 1], None, mybir.AluOpType.is_lt)
                    nc.vector.tensor_tensor(
                        u_row[0:1, n * 512:(n + 1) * 512], uf[:], mrow[:],
                        mybir.AluOpType.mult)
                    # broadcast u to all partitions: ones_col ⊗ u_row
                    ps_bc = psB.tile([128, 512], f32)
                    nc.tensor.matmul(ps_bc[:], onesr_sb[:],
                                     u_row[0:1, n * 512:(n + 1) * 512],
                                     start=True, stop=True)
                    nc.scalar.copy(
                        ubc[:, n * 4:(n + 1) * 4, :], ps_bc[:])
                # U contribution of this chunk
                nc.vector.tensor_reduce(Usc[0:1, h:h + 1], u_row[:],
                                        mybir.AxisListType.X,
                                        mybir.AluOpType.add)

                # 6. ctxT[:, h, j] = sum_s encT[:, :, j, :] * ubc
                # (tensor_tensor_reduce is broken on this runtime: two-pass)
                if h == 0:
                    ctxT_acc = smallp.tile([128, NH, NJ], f32, tag="ctxT")
                for j in range(NJ):
                    trash = dumpp.tile([128, NT, 128], bf16, tag="trash")
                    nc.vector.tensor_tensor(trash[:], encT[:, :, j, :],
                                            ubc[:], mybir.AluOpType.mult)
                    nc.vector.tensor_reduce(ctxT_acc[:, h, j:j + 1], trash[:],
                                            mybir.AxisListType.XY,
                                            mybir.AluOpType.add)

            # 7. finalize: U total, broadcast 1/U, scale, store
            utot = smallp.tile([1, 1], f32, tag="utot")
            nc.vector.tensor_reduce(utot[:], Usc[:], mybir.AxisListType.X,
                                    mybir.AluOpType.add)
            rU = smallp.tile([1, 1], f32, tag="rU")
            nc.vector.reciprocal(rU[:], utot[:])
            ps_rb = psS.tile([128, 1], f32, tag="ps_s")
            nc.tensor.matmul(ps_rb[:], onesrf_sb[:], rU[:],
                             start=True, stop=True)
            rU128 = smallp.tile([128, 1], f32, tag="rU128")
            nc.vector.tensor_copy(rU128[:], ps_rb[:])
            ctx_sum = smallp.tile([128, NJ], f32, tag="ctx_sum")
            nc.vector.tensor_tensor(ctx_sum[:], ctxT_acc[:, 0, :],
                                    ctxT_acc[:, 1, :], mybir.AluOpType.add)
            ctx_out = smallp.tile([128, NJ], f32, tag="ctx_out")
            nc.vector.tensor_scalar(ctx_out[:], ctx_sum[:], rU128[:], None,
                                    mybir.AluOpType.mult)
            # transpose [128, NJ] -> [NJ, 128] so the store is contiguous
            ps_ctr = psS.tile([NJ, 128], f32, tag="ps_s")
            nc.tensor.transpose(ps_ctr[:], ctx_out[:], identf_sb[:])
            ctx_row = smallp.tile([NJ, 128], f32, tag="ctx_row")
            nc.scalar.copy(ctx_row[:], ps_ctr[:])
            nc.sync.dma_start(
                out_d.ap()[b:b + 1, :].rearrange("o (j p) -> (o j) p", p=128),
                ctx_row[:])

    nc.compile()
    return nc


def make_in_maps(encoder_out, hidden, lengths, attn_w, attn_b, v_w):
    encoder_out = np.ascontiguousarray(np.asarray(encoder_out, dtype=np.float32))
    hidden = np.asarray(hidden, dtype=np.float32)
    lengths = np.asarray(lengths).astype(np.float32)
    attn_w = np.asarray(attn_w, dtype=np.float32)
    attn_b = np.asarray(attn_b, dtype=np.float32)
    v_w = np.asarray(v_w, dtype=np.float32)

    weT = np.ascontiguousarray(attn_w[:, D:].T).astype(ml_dtypes.bfloat16)
    whT = np.ascontiguousarray(attn_w[:, :D].T)
    vw = v_w[0].astype(ml_dtypes.bfloat16)
    iota_row = np.arange(S, dtype=np.float32).reshape(1, S)
    ones_row = np.ones((1, 128), dtype=ml_dtypes.bfloat16)
    ident = np.eye(128, dtype=ml_dtypes.bfloat16)

    in_maps = []
    for i in range(NCORES):
        sl = slice(i * BL, (i + 1) * BL)
        in_maps.append({
            "enc": np.ascontiguousarray(encoder_out[sl]),
            "weT": weT,
            "whT": whT,
            "hidT": np.ascontiguousarray(hidden[sl].T),
            "battn": attn_b,
            "vw": vw,
            "lens": np.ascontiguousarray(lengths[sl].reshape(1, BL)),
            "iota_row": iota_row,
            "ones_row": ones_row,
            "ones_row_f": np.ones((1, 128), dtype=np.float32),
            "ident": ident,
            "identf": np.eye(128, dtype=np.float32),
        })
    return in_maps


_cached_nc = None


def kernel(encoder_out, hidden, lengths, attn_w, attn_b, v_w, v_b):
    global _cached_nc
    from concourse.bass_utils import run_bass_kernel_spmd

    if _cached_nc is None:
        _cached_nc = build_nc()
    in_maps = make_in_maps(encoder_out, hidden, lengths, attn_w, attn_b, v_w)
    res = run_bass_kernel_spmd(_cached_nc, in_maps, core_ids=list(range(NCORES)))
    out = np.concatenate([res.results[i]["out"] for i in range(NCORES)], axis=0)
    return out.astype(np.float32)


# revision 24
# speedup vs baseline: 1.0629x; 1.0427x over previous
"""Bahdanau-style ragged-sequence attention on 8 TRN2 NeuronCores.

Data-parallel over batch: each core owns 4 batches of
encoder_out [4, 2048, 1024], computes masked-softmax attention context
[4, 1024]; host concatenates the 8 shards.

Per-core pipeline (per batch b, per s-half h of 1024 positions):
  1. SWDGE cast-DMA: enc[b, h] f32 HBM -> bf16 SBUF [128, 8, 1024] (s=t*128+p)
  2. xbar DMA-transpose: -> encT bf16 [128(f%128), 8(f//128), 8(t), 128(q)]
  3. PE: M^T[d, s] = sum_f W_e[d, f] enc[s, f]  (bf16, fp32 PSUM accum)
     ACT: e^T = tanh(M^T + c_b)  (c_b = W_h h_b + attn_b, precomputed on PE)
  4. PE: scoresT[s-slice, 1] = e^T_slice.T @ v  -> scores laid [128, 8]
  5. ACT exp (no max-sub needed: |score| <= |v|_1 ~ 12), DVE mask -> u bf16
  6. PE: ctx_unnorm[1, 1024] += u_t.T @ enc_t ; U[1,1] += u_t.T @ ones
  7. DVE: ctx = ctx_unnorm / U -> DMA out.

v_b is skipped: softmax is invariant to a uniform logit shift, and masked
positions are exact zeros via the mask multiply.
"""
import sys

sys.path.insert(0, '/opt/trn_rl_repo')

from contextlib import ExitStack

import numpy as np
import ml_dtypes

import concourse.bass as bass
import concourse.tile as tile
from concourse import bacc, mybir

B, S, F, D = 32, 2048, 1024, 512
NCORES = 8
BL = B // NCORES          # batches per core
NH = 2                    # s-chunks per batch
SH = S // NH              # 1024 positions per half
NT = SH // 128            # 8 s-tiles per half
NJ = F // 128             # 8 f-chunks
ND = D // 128             # 4 d-chunks
NSPAN = SH // 512         # 2 matmul spans of 512 per half

f32 = mybir.dt.float32
bf16 = mybir.dt.bfloat16


def build_nc():
    nc = bacc.Bacc('TRN2', target_bir_lowering=False, debug=False)

    enc_d = nc.dram_tensor("enc", [BL, S, F], f32, kind="ExternalInput")
    weT_d = nc.dram_tensor("weT", [F, D], bf16, kind="ExternalInput")
    whT_d = nc.dram_tensor("whT", [D, D], f32, kind="ExternalInput")
    hidT_d = nc.dram_tensor("hidT", [D, BL], f32, kind="ExternalInput")
    battn_d = nc.dram_tensor("battn", [D], f32, kind="ExternalInput")
    vw_d = nc.dram_tensor("vw", [D], bf16, kind="ExternalInput")
    lens_d = nc.dram_tensor("lens", [128, BL], f32, kind="ExternalInput")
    iota_d = nc.dram_tensor("iota", [128, NH * NT], f32, kind="ExternalInput")
    ones_d = nc.dram_tensor("ones", [128, 1], f32, kind="ExternalInput")
    out_d = nc.dram_tensor("out", [BL, F], f32, kind="ExternalOutput")

    with tile.TileContext(nc) as tc, ExitStack() as ctx:
        wpool = ctx.enter_context(tc.tile_pool(name="weights", bufs=1))
        encp = ctx.enter_context(tc.tile_pool(name="encp", bufs=4))
        enctp = ctx.enter_context(tc.tile_pool(name="enctp", bufs=3))
        etp = ctx.enter_context(tc.tile_pool(name="etp", bufs=3))
        smallp = ctx.enter_context(tc.tile_pool(name="small", bufs=4))
        psM = ctx.enter_context(tc.tile_pool(name="psM", bufs=2, space="PSUM"))
        psS = ctx.enter_context(tc.tile_pool(name="psS", bufs=1, space="PSUM"))
        psC = ctx.enter_context(tc.tile_pool(name="psC", bufs=2, space="PSUM"))
        psU = ctx.enter_context(tc.tile_pool(name="psU", bufs=1, space="PSUM"))

        # ---- one-time setup ----
        weT = wpool.tile([128, NJ, D], bf16)
        nc.sync.dma_start(weT[:], weT_d.ap().rearrange("(j p) d -> p j d", p=128))
        whT = wpool.tile([128, ND, D], f32)
        nc.sync.dma_start(whT[:], whT_d.ap().rearrange("(j p) d -> p j d", p=128))
        hidT = wpool.tile([128, ND, BL], f32)
        nc.sync.dma_start(hidT[:], hidT_d.ap().rearrange("(j p) b -> p j b", p=128))
        b_sb = wpool.tile([128, ND], f32)
        nc.sync.dma_start(b_sb[:], battn_d.ap().rearrange("(c p) -> p c", p=128))
        v_sb = wpool.tile([128, ND], bf16)
        nc.sync.dma_start(v_sb[:], vw_d.ap().rearrange("(c p) -> p c", p=128))
        len_sb = wpool.tile([128, BL], f32)
        nc.sync.dma_start(len_sb[:], lens_d.ap())
        iota_sb = wpool.tile([128, NH * NT], f32)
        nc.sync.dma_start(iota_sb[:], iota_d.ap())
        ones_sb = wpool.tile([128, 1], f32)
        nc.sync.dma_start(ones_sb[:], ones_d.ap())

        # c[d, b] = (W_h @ h_b + attn_b) laid [128, dc, b]
        ps_c = psS.tile([128, ND * BL], f32, tag="ps_s")
        for dc in range(ND):
            for j in range(ND):
                nc.tensor.matmul(
                    ps_c[:, dc * BL:(dc + 1) * BL],
                    whT[:, j, dc * 128:(dc + 1) * 128],
                    hidT[:, j, :],
                    start=(j == 0), stop=(j == ND - 1))
        c_sb = wpool.tile([128, ND, BL], f32)
        for dc in range(ND):
            nc.scalar.activation(
                c_sb[:, dc, :], ps_c[:, dc * BL:(dc + 1) * BL],
                mybir.ActivationFunctionType.Identity, bias=b_sb[:, dc:dc + 1])

        # ---- main loop ----
        for b in range(BL):
            ctx_ps = None
            u_ps = None
            # 1.+2. batch the cast-loads, then the xbar transposes, so the
            # DMA stream switches xbar mode twice per batch instead of per
            # chunk (each DMACopy<->DMATranspose transition serializes).
            encs = []
            encTs = []
            for h in range(NH):
                enc_bf = encp.tile([128, NT, F], bf16)
                src = enc_d.ap()[b, h * SH:(h + 1) * SH, :]
                nc.gpsimd.dma_start(
                    enc_bf[:], src.rearrange("(t p) f -> p t f", p=128))
                encs.append(enc_bf)
            for h in range(NH):
                # encT[p, t, j, q] = enc[t*128+q, j*128+p]
                encT = enctp.tile([128, NT, NJ, 128], bf16)
                nc.sync.dma_start_transpose(encT[:, :, :, :], encs[h][:, :, :])
                encTs.append(encT)
            for h in range(NH):
                enc_bf = encs[h]
                encT = encTs[h]
                # 3. scores matmul + tanh -> eT [128, ND, SH] bf16
                eT = etp.tile([128, ND, SH], bf16)
                for n in range(NSPAN):
                    tsl = slice(n * 4, (n + 1) * 4)
                    for dc in range(ND):
                        pm = psM.tile([128, 512], f32)
                        for j in range(NJ):
                            nc.tensor.matmul(
                                pm[:],
                                weT[:, j, dc * 128:(dc + 1) * 128],
                                encT[:, tsl, j, :],
                                start=(j == 0), stop=(j == NJ - 1))
                        nc.scalar.activation(
                            eT[:, dc, n * 512:(n + 1) * 512], pm[:],
                            mybir.ActivationFunctionType.Tanh,
                            bias=c_sb[:, dc, b:b + 1])

                # 4. scoresT [128, NT]: col t = e^T slice.T @ v
                ps_s = psS.tile([128, NT], f32, tag="ps_s")
                for t in range(NT):
                    for dc in range(ND):
                        nc.tensor.matmul(
                            ps_s[:, t:t + 1],
                            eT[:, dc, t * 128:(t + 1) * 128],
                            v_sb[:, dc:dc + 1],
                            start=(dc == 0), stop=(dc == ND - 1))

                # 5. u = exp(scores) * mask  (bf16)
                mask = smallp.tile([128, NT], f32, tag="mask")
                nc.vector.tensor_scalar(
                    mask[:], iota_sb[:, h * NT:(h + 1) * NT],
                    len_sb[:, b:b + 1], None, mybir.AluOpType.is_lt)
                u_f = smallp.tile([128, NT], f32, tag="uf")
                nc.scalar.activation(u_f[:], ps_s[:],
                                     mybir.ActivationFunctionType.Exp)
                u_bf = smallp.tile([128, NT], bf16, tag="ubf")
                nc.vector.tensor_tensor(u_bf[:], u_f[:], mask[:],
                                        mybir.AluOpType.mult)

                # 6. context accumulation across both halves; U via DVE
                if h == 0:
                    ctx_ps = psC.tile([1, F], f32)
                    u_ps = psU.tile([1, 1], f32)
                    us2 = smallp.tile([128, NH], f32, tag="us2")
                nc.vector.tensor_reduce(us2[:, h:h + 1], u_bf[:],
                                        mybir.AxisListType.X,
                                        mybir.AluOpType.add)
                for t in range(NT):
                    first = (h == 0 and t == 0)
                    last = (h == NH - 1 and t == NT - 1)
                    nc.tensor.matmul(ctx_ps[0:1, 0:512], u_bf[:, t:t + 1],
                                     enc_bf[:, t, 0:512],
                                     start=first, stop=last,
                                     skip_group_check=True)
                    nc.tensor.matmul(ctx_ps[0:1, 512:1024], u_bf[:, t:t + 1],
                                     enc_bf[:, t, 512:1024],
                                     start=first, stop=last,
                                     skip_group_check=True)

            # 7. finalize batch: U = sum over partitions of (us2[:,0]+us2[:,1])
            uscol = smallp.tile([128, 1], f32, tag="uscol")
            nc.vector.tensor_reduce(uscol[:], us2[:],
                                    mybir.AxisListType.X,
                                    mybir.AluOpType.add)
            nc.tensor.matmul(u_ps[:], uscol[:], ones_sb[:], start=True, stop=True)
            rU = smallp.tile([1, 1], f32, tag="rU")
            nc.vector.reciprocal(rU[:], u_ps[0:1, 0:1])
            ctx_sb = smallp.tile([1, F], f32, tag="ctx")
            nc.vector.tensor_scalar(ctx_sb[:], ctx_ps[:], rU[0:1, 0:1], None,
                                    mybir.AluOpType.mult)
            nc.sync.dma_start(out_d.ap()[b:b + 1, :], ctx_sb[:])

    nc.compile()
    return nc


def make_in_maps(encoder_out, hidden, lengths, attn_w, attn_b, v_w):
    encoder_out = np.ascontiguousarray(np.asarray(encoder_out, dtype=np.float32))
    hidden = np.asarray(hidden, dtype=np.float32)
    lengths = np.asarray(lengths).astype(np.float32)
    attn_w = np.asarray(attn_w, dtype=np.float32)
    attn_b = np.asarray(attn_b, dtype=np.float32)
    v_w = np.asarray(v_w, dtype=np.float32)

    weT = np.ascontiguousarray(attn_w[:, D:].T).astype(ml_dtypes.bfloat16)
    whT = np.ascontiguousarray(attn_w[:, :D].T)
    vw = v_w[0].astype(ml_dtypes.bfloat16)
    iota = np.ascontiguousarray(
        np.arange(S, dtype=np.float32).reshape(NH * NT, 128).T)
    ones = np.ones((128, 1), dtype=np.float32)

    in_maps = []
    for i in range(NCORES):
        sl = slice(i * BL, (i + 1) * BL)
        in_maps.append({
            "enc": np.ascontiguousarray(encoder_out[sl]),
            "weT": weT,
            "whT": whT,
            "hidT": np.ascontiguousarray(hidden[sl].T),
            "battn": attn_b,
            "vw": vw,
            "lens": np.ascontiguousarray(
                np.broadcast_to(lengths[sl], (128, BL))),
            "iota": iota,
            "ones": ones,
        })
    return in_maps


_cached_nc = None


def kernel(encoder_out, hidden, lengths, attn_w, attn_b, v_w, v_b):
    global _cached_nc
    from concourse.bass_utils import run_bass_kernel_spmd

    if _cached_nc is None:
        _cached_nc = build_nc()
    in_maps = make_in_maps(encoder_out, hidden, lengths, attn_w, attn_b, v_w)
    res = run_bass_kernel_spmd(_cached_nc, in_maps, core_ids=list(range(NCORES)))
    out = np.concatenate([res.results[i]["out"] for i in range(NCORES)], axis=0)
    return out.astype(np.float32)


# revision 25
# speedup vs baseline: 1.2384x; 1.1651x over previous
"""Bahdanau-style ragged-sequence attention on 8 TRN2 NeuronCores.

Data-parallel over batch: each core owns 4 batches of
encoder_out [4, 2048, 1024], computes masked-softmax attention context
[4, 1024]; host concatenates the 8 shards.

Per-core pipeline (per batch b, per s-half h of 1024 positions):
  1. SWDGE cast-DMA: enc[b, h] f32 HBM -> bf16 SBUF [128, 8, 1024] (s=t*128+p)
  2. xbar DMA-transpose: -> encT bf16 [128(f%128), 8(f//128), 8(t), 128(q)]
  3. PE: M^T[d, s] = sum_f W_e[d, f] enc[s, f]  (bf16, fp32 PSUM accum)
     ACT: e^T = tanh(M^T + c_b)  (c_b = W_h h_b + attn_b, precomputed on PE)
  4. PE: scoresT[s-slice, 1] = e^T_slice.T @ v  -> scores laid [128, 8]
  5. ACT exp (no max-sub needed: |score| <= |v|_1 ~ 12), DVE mask -> u bf16
  6. PE: ctx_unnorm[1, 1024] += u_t.T @ enc_t ; U[1,1] += u_t.T @ ones
  7. DVE: ctx = ctx_unnorm / U -> DMA out.

v_b is skipped: softmax is invariant to a uniform logit shift, and masked
positions are exact zeros via the mask multiply.
"""
import sys

sys.path.insert(0, '/opt/trn_rl_repo')

from contextlib import ExitStack

import numpy as np
import ml_dtypes

import concourse.bass as bass
import concourse.tile as tile
from concourse import bacc, mybir

B, S, F, D = 32, 2048, 1024, 512
NCORES = 8
BL = B // NCORES          # batches per core
NH = 2                    # s-chunks per batch
SH = S // NH              # 1024 positions per half
NT = SH // 128            # 8 s-tiles per half
NJ = F // 128             # 8 f-chunks
ND = D // 128             # 4 d-chunks
NSPAN = SH // 512         # 2 matmul spans of 512 per half
XB_T = 5                  # s-tiles per chunk via xbar DMA; rest on PE

f32 = mybir.dt.float32
bf16 = mybir.dt.bfloat16


def build_nc():
    nc = bacc.Bacc('TRN2', target_bir_lowering=False, debug=False)

    enc_d = nc.dram_tensor("enc", [BL, S, F], f32, kind="ExternalInput")
    weT_d = nc.dram_tensor("weT", [F, D], bf16, kind="ExternalInput")
    whT_d = nc.dram_tensor("whT", [D, D], f32, kind="ExternalInput")
    hidT_d = nc.dram_tensor("hidT", [D, BL], f32, kind="ExternalInput")
    battn_d = nc.dram_tensor("battn", [D], f32, kind="ExternalInput")
    vw_d = nc.dram_tensor("vw", [D], bf16, kind="ExternalInput")
    lens_d = nc.dram_tensor("lens", [128, BL], f32, kind="ExternalInput")
    iota_d = nc.dram_tensor("iota", [128, NH * NT], f32, kind="ExternalInput")
    ones_d = nc.dram_tensor("ones", [128, 1], f32, kind="ExternalInput")
    ident_d = nc.dram_tensor("ident", [128, 128], bf16, kind="ExternalInput")
    out_d = nc.dram_tensor("out", [BL, F], f32, kind="ExternalOutput")

    with tile.TileContext(nc) as tc, ExitStack() as ctx:
        wpool = ctx.enter_context(tc.tile_pool(name="weights", bufs=1))
        encp = ctx.enter_context(tc.tile_pool(name="encp", bufs=4))
        enctp = ctx.enter_context(tc.tile_pool(name="enctp", bufs=3))
        etp = ctx.enter_context(tc.tile_pool(name="etp", bufs=3))
        smallp = ctx.enter_context(tc.tile_pool(name="small", bufs=4))
        psM = ctx.enter_context(tc.tile_pool(name="psM", bufs=2, space="PSUM"))
        psS = ctx.enter_context(tc.tile_pool(name="psS", bufs=1, space="PSUM"))
        psC = ctx.enter_context(tc.tile_pool(name="psC", bufs=1, space="PSUM"))
        psU = ctx.enter_context(tc.tile_pool(name="psU", bufs=1, space="PSUM"))
        psT = ctx.enter_context(tc.tile_pool(name="psT", bufs=2, space="PSUM"))

        # ---- one-time setup ----
        weT = wpool.tile([128, NJ, D], bf16)
        nc.sync.dma_start(weT[:], weT_d.ap().rearrange("(j p) d -> p j d", p=128))
        whT = wpool.tile([128, ND, D], f32)
        nc.sync.dma_start(whT[:], whT_d.ap().rearrange("(j p) d -> p j d", p=128))
        hidT = wpool.tile([128, ND, BL], f32)
        nc.sync.dma_start(hidT[:], hidT_d.ap().rearrange("(j p) b -> p j b", p=128))
        b_sb = wpool.tile([128, ND], f32)
        nc.sync.dma_start(b_sb[:], battn_d.ap().rearrange("(c p) -> p c", p=128))
        v_sb = wpool.tile([128, ND], bf16)
        nc.sync.dma_start(v_sb[:], vw_d.ap().rearrange("(c p) -> p c", p=128))
        len_sb = wpool.tile([128, BL], f32)
        nc.sync.dma_start(len_sb[:], lens_d.ap())
        iota_sb = wpool.tile([128, NH * NT], f32)
        nc.sync.dma_start(iota_sb[:], iota_d.ap())
        ones_sb = wpool.tile([128, 1], f32)
        nc.sync.dma_start(ones_sb[:], ones_d.ap())
        ident_sb = wpool.tile([128, 128], bf16)
        nc.sync.dma_start(ident_sb[:], ident_d.ap())

        # c[d, b] = (W_h @ h_b + attn_b) laid [128, dc, b]
        ps_c = psS.tile([128, ND * BL], f32, tag="ps_s")
        for dc in range(ND):
            for j in range(ND):
                nc.tensor.matmul(
                    ps_c[:, dc * BL:(dc + 1) * BL],
                    whT[:, j, dc * 128:(dc + 1) * 128],
                    hidT[:, j, :],
                    start=(j == 0), stop=(j == ND - 1))
        c_sb = wpool.tile([128, ND, BL], f32)
        for dc in range(ND):
            nc.scalar.activation(
                c_sb[:, dc, :], ps_c[:, dc * BL:(dc + 1) * BL],
                mybir.ActivationFunctionType.Identity, bias=b_sb[:, dc:dc + 1])

        # ---- main loop ----
        for b in range(BL):
            ctx_ps = None
            u_ps = None
            # 1.+2. batch the cast-loads, then the xbar transposes, so the
            # DMA stream switches xbar mode twice per batch instead of per
            # chunk (each DMACopy<->DMATranspose transition serializes).
            encs = []
            encTs = []
            for h in range(NH):
                enc_bf = encp.tile([128, NT, F], bf16)
                src = enc_d.ap()[b, h * SH:(h + 1) * SH, :]
                nc.gpsimd.dma_start(
                    enc_bf[:], src.rearrange("(t p) f -> p t f", p=128))
                encs.append(enc_bf)
            for h in range(NH):
                # encT[p, t, j, q] = enc[t*128+q, j*128+p]
                encT = enctp.tile([128, NT, NJ, 128], bf16)
                nc.sync.dma_start_transpose(encT[:, 0:XB_T, :, :],
                                            encs[h][:, 0:XB_T, :])
                encTs.append(encT)
            for h in range(NH):
                # remaining s-tiles transposed on PE, evacuated on ACT/DVE
                for t in range(XB_T, NT):
                    for j in range(NJ):
                        ptr = psT.tile([128, 128], bf16, tag="ptr")
                        nc.tensor.transpose(
                            ptr[:], encs[h][:, t, j * 128:(j + 1) * 128],
                            ident_sb[:])
                        if j % 2 == 0:
                            nc.scalar.copy(encTs[h][:, t, j, :], ptr[:])
                        else:
                            nc.vector.tensor_copy(encTs[h][:, t, j, :], ptr[:])
            for h in range(NH):
                enc_bf = encs[h]
                encT = encTs[h]
                # 3. scores matmul + tanh -> eT [128, ND, SH] bf16
                eT = etp.tile([128, ND, SH], bf16)
                for n in range(NSPAN):
                    tsl = slice(n * 4, (n + 1) * 4)
                    for dc in range(ND):
                        pm = psM.tile([128, 512], f32)
                        for j in range(NJ):
                            nc.tensor.matmul(
                                pm[:],
                                weT[:, j, dc * 128:(dc + 1) * 128],
                                encT[:, tsl, j, :],
                                start=(j == 0), stop=(j == NJ - 1))
                        nc.scalar.activation(
                            eT[:, dc, n * 512:(n + 1) * 512], pm[:],
                            mybir.ActivationFunctionType.Tanh,
                            bias=c_sb[:, dc, b:b + 1])

                # 4. scoresT [128, NT]: col t = e^T slice.T @ v
                ps_s = psS.tile([128, NT], f32, tag="ps_s")
                for t in range(NT):
                    for dc in range(ND):
                        nc.tensor.matmul(
                            ps_s[:, t:t + 1],
                            eT[:, dc, t * 128:(t + 1) * 128],
                            v_sb[:, dc:dc + 1],
                            start=(dc == 0), stop=(dc == ND - 1))

                # 5. u = exp(scores) * mask  (bf16)
                mask = smallp.tile([128, NT], f32, tag="mask")
                nc.vector.tensor_scalar(
                    mask[:], iota_sb[:, h * NT:(h + 1) * NT],
                    len_sb[:, b:b + 1], None, mybir.AluOpType.is_lt)
                u_f = smallp.tile([128, NT], f32, tag="uf")
                nc.scalar.activation(u_f[:], ps_s[:],
                                     mybir.ActivationFunctionType.Exp)
                u_bf = smallp.tile([128, NT], bf16, tag="ubf")
                nc.vector.tensor_tensor(u_bf[:], u_f[:], mask[:],
                                        mybir.AluOpType.mult)

                # 6. context accumulation across both halves; U via DVE
                if h == 0:
                    ctx_ps = psC.tile([1, F], f32)
                    u_ps = psU.tile([1, 1], f32)
                    us2 = smallp.tile([128, NH], f32, tag="us2")
                nc.vector.tensor_reduce(us2[:, h:h + 1], u_bf[:],
                                        mybir.AxisListType.X,
                                        mybir.AluOpType.add)
                for t in range(NT):
                    first = (h == 0 and t == 0)
                    last = (h == NH - 1 and t == NT - 1)
                    nc.tensor.matmul(ctx_ps[0:1, 0:512], u_bf[:, t:t + 1],
                                     enc_bf[:, t, 0:512],
                                     start=first, stop=last,
                                     skip_group_check=True)
                    nc.tensor.matmul(ctx_ps[0:1, 512:1024], u_bf[:, t:t + 1],
                                     enc_bf[:, t, 512:1024],
                                     start=first, stop=last,
                                     skip_group_check=True)

            # 7. finalize batch: U = sum over partitions of (us2[:,0]+us2[:,1])
            uscol = smallp.tile([128, 1], f32, tag="uscol")
            nc.vector.tensor_reduce(uscol[:], us2[:],
                                    mybir.AxisListType.X,
                                    mybir.AluOpType.add)
            nc.tensor.matmul(u_ps[:], uscol[:], ones_sb[:], start=True, stop=True)
            rU = smallp.tile([1, 1], f32, tag="rU")
            nc.vector.reciprocal(rU[:], u_ps[0:1, 0:1])
            ctx_sb = smallp.tile([1, F], f32, tag="ctx")
            nc.vector.tensor_scalar(ctx_sb[:], ctx_ps[:], rU[0:1, 0:1], None,
                                    mybir.AluOpType.mult)
            nc.sync.dma_start(out_d.ap()[b:b + 1, :], ctx_sb[:])

    nc.compile()
    return nc


def make_in_maps(encoder_out, hidden, lengths, attn_w, attn_b, v_w):
    encoder_out = np.ascontiguousarray(np.asarray(encoder_out, dtype=np.float32))
    hidden = np.asarray(hidden, dtype=np.float32)
    lengths = np.asarray(lengths).astype(np.float32)
    attn_w = np.asarray(attn_w, dtype=np.float32)
    attn_b = np.asarray(attn_b, dtype=np.float32)
    v_w = np.asarray(v_w, dtype=np.float32)

    weT = np.ascontiguousarray(attn_w[:, D:].T).astype(ml_dtypes.bfloat16)
    whT = np.ascontiguousarray(attn_w[:, :D].T)
    vw = v_w[0].astype(ml_dtypes.bfloat16)
    iota = np.ascontiguousarray(
        np.arange(S, dtype=np.float32).reshape(NH * NT, 128).T)
    ones = np.ones((128, 1), dtype=np.float32)

    in_maps = []
    for i in range(NCORES):
        sl = slice(i * BL, (i + 1) * BL)
        in_maps.append({
            "enc": np.ascontiguousarray(encoder_out[sl]),
            "weT": weT,
            "whT": whT,
            "hidT": np.ascontiguousarray(hidden[sl].T),
            "battn": attn_b,
            "vw": vw,
            "lens": np.ascontiguousarray(
                np.broadcast_to(lengths[sl], (128, BL))),
            "iota": iota,
            "ones": ones,
            "ident": np.eye(128, dtype=ml_dtypes.bfloat16),
        })
    return in_maps


_cached_nc = None


def kernel(encoder_out, hidden, lengths, attn_w, attn_b, v_w, v_b):
    global _cached_nc
    from concourse.bass_utils import run_bass_kernel_spmd

    if _cached_nc is None:
        _cached_nc = build_nc()
    in_maps = make_in_maps(encoder_out, hidden, lengths, attn_w, attn_b, v_w)
    res = run_bass_kernel_spmd(_cached_nc, in_maps, core_ids=list(range(NCORES)))
    out = np.concatenate([res.results[i]["out"] for i in range(NCORES)], axis=0)
    return out.astype(np.float32)
